# revision 22
# baseline (speedup 1.0000x reference)
"""Affinity module (L2-norm -> gram -> L1 row-norm) on 8 TRN2 cores, v2.

sim = y^T y per batch is SYMMETRIC: each core computes ~51% of its
[2304, 9216] row-slab (diag-block upper-tri tiles, left half of the
dist-1 block, upper-tri of the dist-2 block, bottom row-half of the
dist-3 block) and the host reconstructs the mirrored regions by
transposition while unsharding. L2 normalization is input prep on the
host; the L1 row normalization needs globally-complete rows, so it also
runs on the host during assembly (a device version would force a second
full DMA pass over the output).

Device: fp16 y in (rolled so every core's slab is cols 0:2304 -> one
SPMD NEFF), fp16 raw gram pieces out via f32 PSUM (TRN2 matmul must
write f32 PSUM). PSUM->SBUF staging copies alternate DVE/ACT (2:1, DMA
cannot read PSUM). Flat [128, 85248] fp16 output per core.
"""
import os

import numpy as np

import concourse.bass as bass
import concourse.tile as tile
from concourse import bacc, mybir
from concourse.bass_utils import run_bass_kernel_spmd

B, C, H, W = 2, 512, 96, 96
N = H * W                  # 9216
NCORES = 8
SLABS = 4                  # row-slabs per batch
SLAB = N // SLABS          # 2304
MB = SLAB // 128           # 18 m-blocks per slab
KT = C // 128              # 4 contraction sub-tiles
NT = 512                   # PSUM bank width (f32)
EPS = 1e-12

f32 = mybir.dt.float32
f16 = mybir.dt.float16


def piece_cols(m):
    """Rolled-coord pieces of m-block m: (piece_id, col_start, width)."""
    out = [
        (0, 128 * m, SLAB - 128 * m),
        (1, SLAB, SLAB // 2),
        (2, 2 * SLAB + 128 * m, SLAB - 128 * m),
    ]
    if m >= MB // 2:
        out.append((3, 3 * SLAB, SLAB))
    return out


def strip_layout():
    lay = []
    off = 0
    for m in range(MB):
        for pid, c0, w in piece_cols(m):
            lay.append((m, pid, c0, w, off))
            off += w
    return lay, off


LAYOUT, TOTW = strip_layout()


def _build():
    nc = bacc.Bacc(trn_type="TRN2", num_devices=NCORES)
    y = nc.dram_tensor("y", [C, N], f16, kind="ExternalInput")
    # k-interleaved fast-start copy of cols 0:512 (row p = [k*512 + c]):
    # ONE dma_start (issues serialize at ~650 ns on Sync) delivers all 4
    # k-slices the first K-chain needs, so the PE starts ~2 us earlier.
    y0 = nc.dram_tensor("y0", [128, KT * NT], f16, kind="ExternalInput")
    out = nc.dram_tensor("out", [128, TOTW], f16, kind="ExternalOutput")

    with tile.TileContext(nc) as tc:
        with (
            tc.tile_pool(name="y", bufs=1) as py,
            tc.tile_pool(name="st", bufs=8) as pst,
            tc.tile_pool(name="ps", bufs=8, space="PSUM") as pps,
        ):
            ty0 = py.tile([128, KT * NT], f16, tag="y0", name="y0")
            nc.sync.dma_start(ty0[:], y0[:, :])
            # y resident as 4 k-tiles x 4 col-groups of [128, 2304] fp16
            ytiles = [[None] * 4 for _ in range(KT)]
            for g in range(4):
                for k in range(KT):
                    t = py.tile([128, SLAB], f16, tag=f"y{k}_{g}", name=f"y{k}_{g}")
                    nc.sync.dma_start(
                        t[:], y[k * 128:(k + 1) * 128, g * SLAB:(g + 1) * SLAB]
                    )
                    ytiles[k][g] = t

            cnt = 0

            def emit_strip(m, pid, c0, w, off):
                nonlocal cnt
                st = pst.tile([128, SLAB], f16, tag="st", name=f"st{m}_{pid}")
                for a in range(0, w, NT):
                    cw = min(NT, w - a)
                    g, rel = divmod(c0 + a, SLAB)
                    ps = pps.tile([128, NT], f32, tag="ps", name=f"ps{m}_{pid}_{a}")
                    for k in range(KT):
                        if m == 0 and c0 + a + cw <= NT:
                            # first strip's head chunk: both operands from
                            # the fast-start tile (no wait on the g0 loads)
                            lhsT = ty0[:, k * NT:k * NT + 128]
                            rhs = ty0[:, k * NT + c0 + a:k * NT + c0 + a + cw]
                        else:
                            lhsT = ytiles[k][0][:, 128 * m:128 * (m + 1)]
                            rhs = ytiles[k][g][:, rel:rel + cw]
                        nc.tensor.matmul(
                            ps[:, :cw],
                            lhsT,
                            rhs,
                            start=(k == 0),
                            stop=(k == KT - 1),
                        )
                    # PSUM f32 -> SBUF fp16; only DVE/ACT can read PSUM
                    if cnt % 3 == 2:
                        nc.scalar.copy(st[:, a:a + cw], ps[:, :cw])
                    else:
                        nc.vector.tensor_copy(st[:, a:a + cw], ps[:, :cw])
                    cnt += 1
                nc.sync.dma_start(out[:, off:off + w], st[:, :w])

            # P0 pass first: only needs col-group 0, so compute starts
            # while groups 1-3 stream in; then P1, P2, P3 passes.
            for want in range(4):
                for m, pid, c0, w, off in LAYOUT:
                    if pid == want:
                        emit_strip(m, pid, c0, w, off)

    nc.finalize()
    return nc


_NC = None


def _get_nc():
    global _NC
    if _NC is None:
        _NC = _build()
    return _NC


def normalize_host(x):
    """x [B, C, N] f32 -> y [B, C, N] fp16, L2-normalized over C."""
    l2 = np.sqrt((x * x).sum(axis=1, keepdims=True))
    yn = x / np.maximum(l2, EPS)
    return yn.astype(np.float16)


def assemble(core_outs):
    """core_outs: 8 arrays [128, TOTW] fp16 (core order b*4+s) ->
    [B, N, N] f32 final L1-row-normalized output."""
    res = np.empty((B, N, N), np.float32)
    for b in range(B):
        S = np.empty((N, N), np.float32)
        for s in range(SLABS):
            u = core_outs[b * SLABS + s]
            r0 = s * SLAB
            for m, pid, c0, w, off in LAYOUT:
                a0 = (r0 + c0) % N
                S[r0 + 128 * m:r0 + 128 * (m + 1), a0:a0 + w] = \
                    u[:, off:off + w].astype(np.float32)
        for s in range(SLABS):
            r0 = s * SLAB
            t1 = ((s + 1) % SLABS) * SLAB
            t2 = ((s + 2) % SLABS) * SLAB
            # diag block lower tiles <- upper^T
            for m in range(1, MB):
                S[r0 + 128 * m:r0 + 128 * (m + 1), r0:r0 + 128 * m] = \
                    S[r0:r0 + 128 * m, r0 + 128 * m:r0 + 128 * (m + 1)].T
            # dist-2 block strict-lower tiles <- peer upper^T
            for m in range(1, MB):
                S[r0 + 128 * m:r0 + 128 * (m + 1), t2:t2 + 128 * m] = \
                    S[t2:t2 + 128 * m, r0 + 128 * m:r0 + 128 * (m + 1)].T
            # B_{s,s+1} right half <- (B_{s+1,s} bottom half)^T  [peer P3]
            S[r0:r0 + SLAB, t1 + SLAB // 2:t1 + SLAB] = \
                S[t1 + SLAB // 2:t1 + SLAB, r0:r0 + SLAB].T
            # B_{s+1,s} top half <- (B_{s,s+1} left half)^T      [own P1]
            S[t1:t1 + SLAB // 2, r0:r0 + SLAB] = \
                S[r0:r0 + SLAB, t1:t1 + SLAB // 2].T
        l1 = np.abs(S).sum(axis=1, dtype=np.float64).astype(np.float32)
        res[b] = S / np.maximum(l1, EPS)[:, None]
    return res


def kernel(x: np.ndarray) -> np.ndarray:
    x = np.ascontiguousarray(np.asarray(x), dtype=np.float32)
    assert x.shape == (B, C, H, W), x.shape
    y = normalize_host(x.reshape(B, C, N))
    in_maps = []
    for core in range(NCORES):
        b, s = divmod(core, SLABS)
        yr = np.ascontiguousarray(np.roll(y[b], -s * SLAB, axis=1))
        y0 = np.ascontiguousarray(
            yr[:, :NT].reshape(KT, 128, NT).transpose(1, 0, 2).reshape(128, KT * NT)
        )
        in_maps.append({"y": yr, "y0": y0})

    nc = _get_nc()
    for attempt in range(4):
        try:
            res = run_bass_kernel_spmd(
                nc,
                in_maps,
                core_ids=list(range(NCORES)),
                trace=bool(os.environ.get("AFF_TRACE")),
            )
            break
        except Exception:  # transient device wedge (e.g. NRT_EXEC_UNIT_*)
            if attempt == 3:
                raise
            import time

            time.sleep(15 * (attempt + 1))
    if os.environ.get("AFF_TRACE"):
        kernel.last_exec_time_ns = res.exec_time_ns
        it = getattr(res, "instructions_and_trace", None)
        kernel.last_trace_path = it[1] if it else None

    return assemble([np.asarray(res.results[c]["out"]) for c in range(NCORES)])


# revision 23
# speedup vs baseline: 1.0114x; 1.0114x over previous
"""Affinity module (L2-norm -> gram -> L1 row-norm) on 8 TRN2 cores, v2.

sim = y^T y per batch is SYMMETRIC: each core computes ~51% of its
[2304, 9216] row-slab (diag-block upper-tri tiles, left half of the
dist-1 block, upper-tri of the dist-2 block, bottom row-half of the
dist-3 block) and the host reconstructs the mirrored regions by
transposition while unsharding. L2 normalization is input prep on the
host; the L1 row normalization needs globally-complete rows, so it also
runs on the host during assembly (a device version would force a second
full DMA pass over the output).

Device: fp16 y in (rolled so every core's slab is cols 0:2304 -> one
SPMD NEFF), fp16 raw gram pieces out via f32 PSUM (TRN2 matmul must
write f32 PSUM). PSUM->SBUF staging copies alternate DVE/ACT (2:1, DMA
cannot read PSUM). Flat [128, 85248] fp16 output per core.
"""
import os

import numpy as np

import concourse.bass as bass
import concourse.tile as tile
from concourse import bacc, mybir
from concourse.bass_utils import run_bass_kernel_spmd

B, C, H, W = 2, 512, 96, 96
N = H * W                  # 9216
NCORES = 8
SLABS = 4                  # row-slabs per batch
SLAB = N // SLABS          # 2304
MB = SLAB // 128           # 18 m-blocks per slab
KT = C // 128              # 4 contraction sub-tiles
NT = 512                   # PSUM bank width (f32)
EPS = 1e-12

f32 = mybir.dt.float32
f16 = mybir.dt.float16


def piece_cols(m):
    """Rolled-coord pieces of m-block m: (piece_id, col_start, width)."""
    out = [
        (0, 128 * m, SLAB - 128 * m),
        (1, SLAB, SLAB // 2),
        (2, 2 * SLAB + 128 * m, SLAB - 128 * m),
    ]
    if m >= MB // 2:
        out.append((3, 3 * SLAB, SLAB))
    return out


def strip_layout():
    lay = []
    off = 0
    for m in range(MB):
        for pid, c0, w in piece_cols(m):
            lay.append((m, pid, c0, w, off))
            off += w
    return lay, off


LAYOUT, TOTW = strip_layout()


def _build():
    nc = bacc.Bacc(trn_type="TRN2", num_devices=NCORES)
    y = nc.dram_tensor("y", [C, N], f16, kind="ExternalInput")
    # k-interleaved fast-start copy of cols 0:512 (row p = [k*512 + c]):
    # ONE dma_start (issues serialize at ~650 ns on Sync) delivers all 4
    # k-slices the first K-chain needs, so the PE starts ~2 us earlier.
    y0 = nc.dram_tensor("y0", [128, KT * NT], f16, kind="ExternalInput")
    out = nc.dram_tensor("out", [128, TOTW], f16, kind="ExternalOutput")

    with tile.TileContext(nc) as tc:
        with (
            tc.tile_pool(name="y", bufs=1) as py,
            tc.tile_pool(name="st", bufs=8) as pst,
            tc.tile_pool(name="ps", bufs=8, space="PSUM") as pps,
        ):
            ty0 = py.tile([128, KT * NT], f16, tag="y0", name="y0")
            nc.sync.dma_start(ty0[:], y0[:, :])
            # y resident as 4 k-tiles x 4 col-groups of [128, 2304] fp16
            ytiles = [[None] * 4 for _ in range(KT)]
            for g in range(4):
                for k in range(KT):
                    t = py.tile([128, SLAB], f16, tag=f"y{k}_{g}", name=f"y{k}_{g}")
                    nc.sync.dma_start(
                        t[:], y[k * 128:(k + 1) * 128, g * SLAB:(g + 1) * SLAB]
                    )
                    ytiles[k][g] = t

            cnt = 0

            def emit_strip(m, pid, c0, w, off):
                nonlocal cnt
                st = pst.tile([128, SLAB], f16, tag="st", name=f"st{m}_{pid}")
                for a in range(0, w, NT):
                    cw = min(NT, w - a)
                    g, rel = divmod(c0 + a, SLAB)
                    ps = pps.tile([128, NT], f32, tag="ps", name=f"ps{m}_{pid}_{a}")
                    for k in range(KT):
                        if m == 0 and c0 + a + cw <= NT:
                            # first strip's head chunk: both operands from
                            # the fast-start tile (no wait on the g0 loads)
                            lhsT = ty0[:, k * NT:k * NT + 128]
                            rhs = ty0[:, k * NT + c0 + a:k * NT + c0 + a + cw]
                        else:
                            lhsT = ytiles[k][0][:, 128 * m:128 * (m + 1)]
                            rhs = ytiles[k][g][:, rel:rel + cw]
                        nc.tensor.matmul(
                            ps[:, :cw],
                            lhsT,
                            rhs,
                            start=(k == 0),
                            stop=(k == KT - 1),
                        )
                    # PSUM f32 -> SBUF fp16 on DVE only (115 us total,
                    # fits under the 148 us PE stream). Single producer
                    # per staging slot -> fewer cross-engine semaphores
                    # for the serial epilogue teardown to clear.
                    nc.vector.tensor_copy(st[:, a:a + cw], ps[:, :cw])
                    cnt += 1
                nc.sync.dma_start(out[:, off:off + w], st[:, :w])

            # P0 pass first: only needs col-group 0, so compute starts
            # while groups 1-3 stream in; then P1, P2, P3 passes.
            for want in range(4):
                for m, pid, c0, w, off in LAYOUT:
                    if pid == want:
                        emit_strip(m, pid, c0, w, off)

    nc.finalize()
    return nc


_NC = None


def _get_nc():
    global _NC
    if _NC is None:
        _NC = _build()
    return _NC


def normalize_host(x):
    """x [B, C, N] f32 -> y [B, C, N] fp16, L2-normalized over C."""
    l2 = np.sqrt((x * x).sum(axis=1, keepdims=True))
    yn = x / np.maximum(l2, EPS)
    return yn.astype(np.float16)


def assemble(core_outs):
    """core_outs: 8 arrays [128, TOTW] fp16 (core order b*4+s) ->
    [B, N, N] f32 final L1-row-normalized output."""
    res = np.empty((B, N, N), np.float32)
    for b in range(B):
        S = np.empty((N, N), np.float32)
        for s in range(SLABS):
            u = core_outs[b * SLABS + s]
            r0 = s * SLAB
            for m, pid, c0, w, off in LAYOUT:
                a0 = (r0 + c0) % N
                S[r0 + 128 * m:r0 + 128 * (m + 1), a0:a0 + w] = \
                    u[:, off:off + w].astype(np.float32)
        for s in range(SLABS):
            r0 = s * SLAB
            t1 = ((s + 1) % SLABS) * SLAB
            t2 = ((s + 2) % SLABS) * SLAB
            # diag block lower tiles <- upper^T
            for m in range(1, MB):
                S[r0 + 128 * m:r0 + 128 * (m + 1), r0:r0 + 128 * m] = \
                    S[r0:r0 + 128 * m, r0 + 128 * m:r0 + 128 * (m + 1)].T
            # dist-2 block strict-lower tiles <- peer upper^T
            for m in range(1, MB):
                S[r0 + 128 * m:r0 + 128 * (m + 1), t2:t2 + 128 * m] = \
                    S[t2:t2 + 128 * m, r0 + 128 * m:r0 + 128 * (m + 1)].T
            # B_{s,s+1} right half <- (B_{s+1,s} bottom half)^T  [peer P3]
            S[r0:r0 + SLAB, t1 + SLAB // 2:t1 + SLAB] = \
                S[t1 + SLAB // 2:t1 + SLAB, r0:r0 + SLAB].T
            # B_{s+1,s} top half <- (B_{s,s+1} left half)^T      [own P1]
            S[t1:t1 + SLAB // 2, r0:r0 + SLAB] = \
                S[r0:r0 + SLAB, t1:t1 + SLAB // 2].T
        l1 = np.abs(S).sum(axis=1, dtype=np.float64).astype(np.float32)
        res[b] = S / np.maximum(l1, EPS)[:, None]
    return res


def kernel(x: np.ndarray) -> np.ndarray:
    x = np.ascontiguousarray(np.asarray(x), dtype=np.float32)
    assert x.shape == (B, C, H, W), x.shape
    y = normalize_host(x.reshape(B, C, N))
    in_maps = []
    for core in range(NCORES):
        b, s = divmod(core, SLABS)
        yr = np.ascontiguousarray(np.roll(y[b], -s * SLAB, axis=1))
        y0 = np.ascontiguousarray(
            yr[:, :NT].reshape(KT, 128, NT).transpose(1, 0, 2).reshape(128, KT * NT)
        )
        in_maps.append({"y": yr, "y0": y0})

    nc = _get_nc()
    for attempt in range(4):
        try:
            res = run_bass_kernel_spmd(
                nc,
                in_maps,
                core_ids=list(range(NCORES)),
                trace=bool(os.environ.get("AFF_TRACE")),
            )
            break
        except Exception:  # transient device wedge (e.g. NRT_EXEC_UNIT_*)
            if attempt == 3:
                raise
            import time

            time.sleep(15 * (attempt + 1))
    if os.environ.get("AFF_TRACE"):
        kernel.last_exec_time_ns = res.exec_time_ns
        it = getattr(res, "instructions_and_trace", None)
        kernel.last_trace_path = it[1] if it else None

    return assemble([np.asarray(res.results[c]["out"]) for c in range(NCORES)])
